# revision 1
# baseline (speedup 1.0000x reference)
"""KA-attention (crossinf) Trainium2 kernel.

Math notes (exact, not approximations):
  reference computes  out = softmax_j( sum_d sigmoid(y_q)[b,h,i,d] + sum_d sigmoid(y_k)[b,h,j,d] )
  The first term is constant along the softmax axis j, so it cancels
  (softmax shift-invariance):  out[b,h,i,j] = softmax_j( B[b,h,j] ),
  B[b,h,j] = sum_d sigmoid(y_k)[b,h,j,d],  y_k = f_q * scale_sp + silu(qf) @ Wq.T.
  Only the q-path (base_weight_q, coef_q) is mathematically needed.

Sharding: tensor-shard the 8192 output rows of base_weight_q across 8 cores
(1024 rows / core, 32 MB of weights per core = the memory roofline).  Each
core computes B for its 64 (h, j) pairs; host gathers the 8x(16,64) partials,
applies the (tiny) softmax and broadcasts over the cancelled i axis.
"""

import sys
import numpy as np

for _p in ("/opt/trn_rl_repo", "/root/.axon_site/_ro/trn_rl_repo"):
    if _p not in sys.path:
        sys.path.append(_p)

import concourse.bass as bass
import concourse.tile as tile
from concourse import bacc, mybir
from concourse.bass_utils import run_bass_kernel_spmd

# Problem shapes (hardcoded per contract)
B, H, P, D = 16, 4, 128, 16
NUM = H * P * D          # 8192
NF = 8                   # spline basis size
NC = 8                   # cores
NS = NUM // NC           # 1024 output rows per core
KT = NUM // 128          # 64 k-tiles of 128
F32 = mybir.dt.float32

# knobs (test.py pokes these)
TRACE = False
TRACE_KW = {}
W_BUFS = 8

_CACHE = {}


# packed "small params" layout: [qs | sc | grid(pad 1024) | coef(8*1024)]
QS_OFF, SC_OFF, GR_OFF, CF_OFF = 0, NS, 2 * NS, 3 * NS
SM_W = 3 * NS + NF * NS  # 11264


def _build_bass():
    nc = bacc.Bacc("TRN2", target_bir_lowering=False, debug=False)
    wt = nc.declare_dram_parameter("wt", [KT, 128, NS], F32, isOutput=False)
    qt = nc.declare_dram_parameter("qt", [128, KT, B], F32, isOutput=False)
    sm = nc.declare_dram_parameter("sm", [B, SM_W], F32, isOutput=False)
    bs = nc.declare_dram_parameter("bs", [B, NS // D], F32, isOutput=True)

    with tile.TileContext(nc) as tc:
        with (
            tc.tile_pool(name="w", bufs=W_BUFS) as wpool,
            tc.tile_pool(name="stat", bufs=1) as stat,
            tc.tile_pool(name="work", bufs=2) as work,
            tc.tile_pool(name="psum", bufs=1, space=bass.MemorySpace.PSUM) as psum,
        ):
            # static (loaded once) tiles
            qt_s = stat.tile([128, KT, B], F32)
            sq = stat.tile([128, KT, B], F32)      # silu(qf).T, k-tiled
            sm_s = stat.tile([B, SM_W], F32)
            sp = stat.tile([B, NS], F32)           # spline accumulator
            y = stat.tile([B, NS], F32)
            sig = stat.tile([B, NS], F32)
            bsum = stat.tile([B, NS // D], F32)

            nc.sync.dma_start(out=qt_s, in_=qt[:, :, :])
            nc.sync.dma_start(out=sm_s, in_=sm[:, :])
            qs_s = sm_s[:, QS_OFF:QS_OFF + NS]
            sc_s = sm_s[:, SC_OFF:SC_OFF + NS]

            # silu(x) = x * sigmoid(x) over the transposed-q block
            sg_t = stat.tile([128, KT, B], F32)
            nc.scalar.activation(sg_t[:, :, :], qt_s[:, :, :],
                                 mybir.ActivationFunctionType.Sigmoid)
            nc.vector.tensor_mul(sq[:, :, :], qt_s[:, :, :], sg_t[:, :, :])

            # KAN sin-basis spline: sp = sum_f cf[:,f,:] * sin(grid_f * qs)
            # ScalarE Sin needs args in [-pi, pi]: Cody-Waite range reduction
            # with round-to-nearest via the fp32 magic-number trick.
            INV2PI = 0.15915494309189535
            MAGIC = 12582912.0            # 1.5 * 2**23
            C1 = 6.28125                  # 2*pi split, c1 exact in fp32
            C2 = 1.9353071e-03            # fp32(2*pi - c1)
            C3 = 8.9833e-11               # remainder
            PI_CLAMP = 3.1415925          # just under fp64 pi
            mm = mybir.AluOpType
            for f in range(NF):
                tf = work.tile([B, NS], F32, tag="tf")
                nc.vector.tensor_scalar_mul(
                    tf, qs_s, sm_s[:, GR_OFF + f:GR_OFF + f + 1])
                kr = work.tile([B, NS], F32, tag="kr")
                nc.vector.tensor_scalar(kr, tf, INV2PI, MAGIC,
                                        op0=mm.mult, op1=mm.add)
                k2 = work.tile([B, NS], F32, tag="k2")
                nc.vector.tensor_scalar_sub(k2, kr, MAGIC)
                red = work.tile([B, NS], F32, tag="red")
                nc.vector.cody_waite_cascade(red, tf, k2, C1, C2, C3)
                redc = work.tile([B, NS], F32, tag="redc")
                nc.vector.tensor_scalar(redc, red, PI_CLAMP, -PI_CLAMP,
                                        op0=mm.min, op1=mm.max)
                sin_t = work.tile([B, NS], F32, tag="sin")
                nc.scalar.activation(sin_t, redc,
                                     mybir.ActivationFunctionType.Sin)
                cf_f = sm_s[:, CF_OFF + f * NS:CF_OFF + (f + 1) * NS]
                if f == 0:
                    nc.vector.tensor_mul(sp, sin_t, cf_f)
                else:
                    prod = work.tile([B, NS], F32, tag="prod")
                    nc.vector.tensor_mul(prod, sin_t, cf_f)
                    nc.vector.tensor_add(sp, sp, prod)
            nc.vector.tensor_mul(sp, sp, sc_s)

            # base: acc[b, n] = sum_k silu(qf)[b, k] * Wq[n0+n, k]
            acc = psum.tile([B, NS], F32)
            for kt in range(KT):
                w_t = wpool.tile([128, NS], F32, tag="w")
                nc.sync.dma_start(out=w_t, in_=wt[kt, :, :])
                for half in range(NS // 512):
                    nc.tensor.matmul(
                        acc[:, half * 512:(half + 1) * 512],
                        sq[:, kt, :],
                        w_t[:, half * 512:(half + 1) * 512],
                        start=(kt == 0),
                        stop=(kt == KT - 1),
                    )

            # y = spline + base ; sig = sigmoid(y) ; B = sum over d-groups
            nc.vector.tensor_add(y, acc[:, :], sp)
            nc.scalar.activation(sig, y, mybir.ActivationFunctionType.Sigmoid)
            nc.vector.reduce_sum(
                out=bsum,
                in_=sig.rearrange("p (j d) -> p j d", d=D),
                axis=mybir.AxisListType.X,
            )
            nc.sync.dma_start(out=bs[:, :], in_=bsum)
    nc.compile()
    return nc


def kernel(q, k, v, grid, base_weight_q, base_weight_k, coef_q, coef_k, scale_sp):
    q = np.asarray(q, dtype=np.float32)
    grid = np.asarray(grid, dtype=np.float32)
    base_weight_q = np.asarray(base_weight_q, dtype=np.float32)
    coef_q = np.asarray(coef_q, dtype=np.float32)
    scale_sp = np.asarray(scale_sp, dtype=np.float32)

    qf = q.reshape(B, NUM)
    # lhsT layout: (128, KT, B) with [kp, kt, b] = qf[b, kt*128 + kp]
    qt = np.ascontiguousarray(qf.T.reshape(KT, 128, B).transpose(1, 0, 2))
    gr = np.ascontiguousarray(np.broadcast_to(grid[None, :], (B, NF)))

    in_maps = []
    for c in range(NC):
        n0 = c * NS
        wt = np.ascontiguousarray(base_weight_q[n0:n0 + NS, :].T).reshape(KT, 128, NS)
        sm = np.zeros((B, SM_W), np.float32)
        sm[:, QS_OFF:QS_OFF + NS] = qf[:, n0:n0 + NS]
        sm[:, SC_OFF:SC_OFF + NS] = scale_sp[None, n0:n0 + NS]
        sm[:, GR_OFF:GR_OFF + NF] = gr
        sm[:, CF_OFF:CF_OFF + NF * NS] = \
            coef_q[n0:n0 + NS, :].T.reshape(1, NF * NS)
        in_maps.append({"wt": wt, "qt": qt, "sm": sm})

    if "nc" not in _CACHE:
        _CACHE["nc"] = _build_bass()
    res = run_bass_kernel_spmd(_CACHE["nc"], in_maps, list(range(NC)),
                               trace=TRACE, **TRACE_KW)
    _CACHE["last_result"] = res

    Bmat = np.empty((B, H, P), np.float32)
    for c in range(NC):
        h, j0 = c // 2, 64 * (c % 2)
        Bmat[:, h, j0:j0 + 64] = res.results[c]["bs"]

    # softmax over j (float32, same stabilized form jax uses)
    m = Bmat.max(axis=-1, keepdims=True)
    e = np.exp(Bmat - m)
    soft = (e / e.sum(axis=-1, keepdims=True)).astype(np.float32)
    return np.ascontiguousarray(
        np.broadcast_to(soft[:, :, None, :], (B, H, P, P)))



# revision 4
# speedup vs baseline: 3.3534x; 3.3534x over previous
"""KA-attention (crossinf) Trainium2 kernel.

Math notes (exact, not approximations):
  reference computes  out = softmax_j( sum_d sigmoid(y_q)[b,h,i,d] + sum_d sigmoid(y_k)[b,h,j,d] )
  The first term is constant along the softmax axis j, so it cancels
  (softmax shift-invariance):  out[b,h,i,j] = softmax_j( B[b,h,j] ),
  B[b,h,j] = sum_d sigmoid(y_k)[b,h,j,d],  y_k = f_q * scale_sp + silu(qf) @ Wq.T.
  Only the q-path (base_weight_q, coef_q) is mathematically needed.

Sharding: tensor-shard the 8192 output rows of base_weight_q across 8 cores
(1024 rows / core).  Each core computes B for its 64 (h, j) pairs; host
gathers the 8x(16,64) partials, applies the (tiny) softmax and broadcasts
over the cancelled i axis.

Performance structure (v2):
  - Weights are quantized host-side to fp8e4 (8 MiB/core instead of 32) and
    streamed as 8 x 1 MiB chunks with 8 KiB partition rows.  The y error
    from fp8 weight+activation quantization is O(+-1) on values of O(850)
    (the matmul accumulates 8192 ~U[0,1)*silu(N(0,1)) products), far inside
    the 2e-2 output tolerance.
  - Matmul runs in DoubleRow fp8 perf mode: 256-deep contraction per
    instruction, 0.5 cycles/row -> PE stream time ~9 us (was 135 us fp32).
  - The KAN sin-spline runs in n-on-partition layout ([128, nt, b] tiles,
    all 128 partitions busy) and is transposed to the matmul's [b, n]
    layout with 8 TensorE transposes at the tail.
  - `yo` (pre-sigmoid y) is also written out; the final softmax output is
    saturation-uniform, so test harnesses should check yo to catch matmul
    or layout bugs.
"""

import sys
import numpy as np

for _p in ("/opt/trn_rl_repo", "/root/.axon_site/_ro/trn_rl_repo"):
    if _p not in sys.path:
        sys.path.append(_p)

import concourse.bass as bass
import concourse.tile as tile
from concourse import bacc, mybir
from concourse.bass_utils import run_bass_kernel_spmd
from concourse.masks import make_identity

# Problem shapes (hardcoded per contract)
B, H, P, D = 16, 4, 128, 16
NUM = H * P * D          # 8192
NF = 8                   # spline basis size
NC = 8                   # cores
NS = NUM // NC           # 1024 output rows per core
NT = NS // 128           # 8 n-tiles of 128 per core
KT = NUM // 128          # 64 k-tiles of 128
NCH = 8                  # weight chunks per core (8 k-tiles each)
KPC = KT // NCH          # k-tiles per chunk = 8 (4 DoubleRow pairs)
F32 = mybir.dt.float32
F8 = mybir.dt.float8e4
NP_F8 = mybir.dt.np(F8)

# knobs (test.py pokes these)
TRACE = False
TRACE_KW = {}
W_BUFS = 4

_CACHE = {}


def _build_bass():
    nc = bacc.Bacc("TRN2", target_bir_lowering=False, debug=False)
    # weights: wt[ch, kp, p4, k2, n] = Wq[n0+n, ((ch*4+p4)*2+k2)*128 + kp], fp8
    wt = nc.declare_dram_parameter("wt", [NCH, 128, KPC // 2, 2, NS], F8, isOutput=False)
    qt = nc.declare_dram_parameter("qt", [128, KT, B], F32, isOutput=False)
    qs = nc.declare_dram_parameter("qs", [128, NT * B], F32, isOutput=False)
    gr = nc.declare_dram_parameter("gr", [128, NF], F32, isOutput=False)
    cf = nc.declare_dram_parameter("cf", [128, NF, NT * B], F32, isOutput=False)
    bs = nc.declare_dram_parameter("bs", [B, NS // D], F32, isOutput=True)
    yo = nc.declare_dram_parameter("yo", [B, NS], F32, isOutput=True)

    with tile.TileContext(nc) as tc:
        with (
            tc.tile_pool(name="w", bufs=W_BUFS) as wpool,
            tc.tile_pool(name="stat", bufs=1) as stat,
            tc.tile_pool(name="work", bufs=2) as work,
            tc.tile_pool(name="psum", bufs=1, space=bass.MemorySpace.PSUM) as psum,
        ):
            # static (loaded once) tiles
            qt_s = stat.tile([128, KT, B], F32)
            sq = stat.tile([128, KT, B], F8)       # silu(qf).T, fp8 lhsT
            qs_s = stat.tile([128, NT * B], F32)
            gr_s = stat.tile([128, NF], F32)
            cf_s = stat.tile([128, NF, NT * B], F32)
            sp = stat.tile([128, NT * B], F32)     # spline accum (n-layout)
            ident = stat.tile([128, 128], F32)
            y = stat.tile([B, NS], F32)
            sig = stat.tile([B, NS], F32)
            bsum = stat.tile([B, NS // D], F32)

            make_identity(nc, ident)
            nc.sync.dma_start(out=qt_s, in_=qt[:, :, :])
            nc.sync.dma_start(out=qs_s, in_=qs[:, :])
            nc.sync.dma_start(out=gr_s, in_=gr[:, :])
            nc.sync.dma_start(out=cf_s, in_=cf[:, :, :])

            # silu(x) in one ScalarE op, writing the fp8 matmul operand
            nc.scalar.activation(sq[:, :, :], qt_s[:, :, :],
                                 mybir.ActivationFunctionType.Silu)

            # base: acc[b, n] = sum_k silu(qf)[b, k] * Wq[n0+n, k]
            # fp8 DoubleRow: each matmul contracts 256 (two k-tiles).
            acc = psum.tile([B, NS], F32)
            spT = psum.tile([B, NS], F32)
            for ch in range(NCH):
                w_t = wpool.tile([128, KPC // 2, 2, NS], F8, tag="w")
                nc.sync.dma_start(out=w_t, in_=wt[ch, :, :, :, :])
                for p4 in range(KPC // 2):
                    kt0 = ch * KPC + p4 * 2
                    for half in range(NS // 512):
                        nc.tensor.matmul(
                            acc[:, half * 512:(half + 1) * 512],
                            sq[:, kt0:kt0 + 2, :],
                            w_t[:, p4, :, half * 512:(half + 1) * 512],
                            start=(ch == 0 and p4 == 0),
                            stop=(ch == NCH - 1 and p4 == KPC // 2 - 1),
                            perf_mode=mybir.MatmulPerfMode.DoubleRow,
                        )

            # KAN sin-basis spline in n-layout: sp[p, nt, b] =
            #   sum_f cf[p, f, nt, b] * sin(grid_f * qs[p, nt, b])
            # ScalarE Sin needs args in [-pi, pi]: Cody-Waite range reduction
            # with round-to-nearest via the fp32 magic-number trick.
            INV2PI = 0.15915494309189535
            MAGIC = 12582912.0            # 1.5 * 2**23
            C1 = 6.28125                  # 2*pi split, c1 exact in fp32
            C2 = 1.9353071e-03            # fp32(2*pi - c1)
            C3 = 8.9833e-11               # remainder
            PI_CLAMP = 3.1415925          # just under fp64 pi
            mm = mybir.AluOpType
            for f in range(NF):
                tf = work.tile([128, NT * B], F32, tag="tf")
                nc.vector.tensor_scalar_mul(tf, qs_s, gr_s[:, f:f + 1])
                kr = work.tile([128, NT * B], F32, tag="kr")
                nc.vector.tensor_scalar(kr, tf, INV2PI, MAGIC,
                                        op0=mm.mult, op1=mm.add)
                k2 = work.tile([128, NT * B], F32, tag="k2")
                nc.vector.tensor_scalar_sub(k2, kr, MAGIC)
                red = work.tile([128, NT * B], F32, tag="red")
                nc.vector.cody_waite_cascade(red, tf, k2, C1, C2, C3)
                redc = work.tile([128, NT * B], F32, tag="redc")
                nc.vector.tensor_scalar(redc, red, PI_CLAMP, -PI_CLAMP,
                                        op0=mm.min, op1=mm.max)
                sin_t = work.tile([128, NT * B], F32, tag="sin")
                nc.scalar.activation(sin_t, redc,
                                     mybir.ActivationFunctionType.Sin)
                if f == 0:
                    nc.vector.tensor_mul(sp, sin_t, cf_s[:, 0])
                else:
                    prod = work.tile([128, NT * B], F32, tag="prod")
                    nc.vector.tensor_mul(prod, sin_t, cf_s[:, f])
                    nc.vector.tensor_add(sp, sp, prod)

            # transpose spline to the matmul layout: spT[b, nt*128+p]
            for nt_i in range(NT):
                nc.tensor.transpose(
                    spT[:, nt_i * 128:(nt_i + 1) * 128],
                    sp[:, nt_i * B:(nt_i + 1) * B], ident)

            # y = base + spline ; sig = sigmoid(y) ; B = sum over d-groups
            # (DVE has a single PSUM read port: stage spT through SBUF)
            sp_b = stat.tile([B, NS], F32)
            nc.vector.tensor_copy(sp_b, spT[:, :])
            nc.vector.tensor_add(y, acc[:, :], sp_b)
            nc.scalar.activation(sig, y, mybir.ActivationFunctionType.Sigmoid)
            nc.vector.reduce_sum(
                out=bsum,
                in_=sig.rearrange("p (j d) -> p j d", d=D),
                axis=mybir.AxisListType.X,
            )
            nc.sync.dma_start(out=bs[:, :], in_=bsum)
            nc.sync.dma_start(out=yo[:, :], in_=y)
    nc.compile()
    return nc


def kernel(q, k, v, grid, base_weight_q, base_weight_k, coef_q, coef_k, scale_sp):
    q = np.asarray(q, dtype=np.float32)
    grid = np.asarray(grid, dtype=np.float32)
    base_weight_q = np.asarray(base_weight_q, dtype=np.float32)
    coef_q = np.asarray(coef_q, dtype=np.float32)
    scale_sp = np.asarray(scale_sp, dtype=np.float32)

    qf = q.reshape(B, NUM)
    # lhsT layout: (128, KT, B) with [kp, kt, b] = qf[b, kt*128 + kp]
    qt = np.ascontiguousarray(qf.T.reshape(KT, 128, B).transpose(1, 0, 2))
    gr = np.ascontiguousarray(np.broadcast_to(grid[None, :], (128, NF)))
    csc = coef_q * scale_sp[:, None]          # fold scale_sp into coef

    in_maps = []
    for c in range(NC):
        n0 = c * NS
        # wt[ch, kp, p4, k2, n] = Wq[n0+n, ((ch*4+p4)*2+k2)*128 + kp]
        wt = base_weight_q[n0:n0 + NS, :].T.reshape(NCH, KPC // 2, 2, 128, NS)
        wt = np.ascontiguousarray(wt.transpose(0, 3, 1, 2, 4)).astype(NP_F8)
        # qs[p, nt, b] = qf[b, n0 + nt*128 + p]
        qs = np.ascontiguousarray(
            qf[:, n0:n0 + NS].T.reshape(NT, 128, B).transpose(1, 0, 2)
        ).reshape(128, NT * B)
        # cf[p, f, nt, b] = csc[n0 + nt*128 + p, f]  (broadcast over b)
        cfc = csc[n0:n0 + NS, :].T.reshape(NF, NT, 128).transpose(2, 0, 1)
        cfb = np.ascontiguousarray(
            np.broadcast_to(cfc[:, :, :, None], (128, NF, NT, B))
        ).reshape(128, NF, NT * B)
        in_maps.append({"wt": wt, "qt": qt, "qs": qs, "gr": gr, "cf": cfb})

    if "nc" not in _CACHE:
        _CACHE["nc"] = _build_bass()
    res = run_bass_kernel_spmd(_CACHE["nc"], in_maps, list(range(NC)),
                               trace=TRACE, **TRACE_KW)
    _CACHE["last_result"] = res

    Bmat = np.empty((B, H, P), np.float32)
    for c in range(NC):
        h, j0 = c // 2, 64 * (c % 2)
        Bmat[:, h, j0:j0 + 64] = res.results[c]["bs"]

    # softmax over j (float32, same stabilized form jax uses)
    m = Bmat.max(axis=-1, keepdims=True)
    e = np.exp(Bmat - m)
    soft = (e / e.sum(axis=-1, keepdims=True)).astype(np.float32)
    return np.ascontiguousarray(
        np.broadcast_to(soft[:, :, None, :], (B, H, P, P)))
